# revision 31
# baseline (speedup 1.0000x reference)
"""Trainium2 Bass kernel for nn_DiagRNN (diagonal complex linear RNN / LRU).

  y = Re[C @ h] + D*x,  h_t = A h_{t-1} + B x_t  (A complex-diagonal)

v4: DVE-centric single-loop structure (rotation, scans, un-rotation all on
DVE with dedicated tiles - no cross-engine gating inside the slice chain),
plus:
  * un-rotation (d = cos*wre - sin*wim) happens in the scan phase,
    carry-independently; the consume-side correction is
    u = d + RC*vp_re - RS*vp_im with RC/RS = r^(k+1)*cos/sin tables.
    z1/z2 are Act scaled-copies at phase tails; zz/u assembly on Pool.
  * D*x folded into the C-GEMM PSUM via a diagonal stationary matmul;
    y epilogue is a plain Act copy.
  * carry pipeline hoisted: gather transposes interleaved into the next
    scanphase's B-GEMM stream; vp ready before the z's run.
"""
import sys, os
sys.path.insert(0, '/opt/trn_rl_repo')
import numpy as np

import concourse.bass as bass
import concourse.bacc as bacc
import concourse.tile as tile
import concourse.mybir as mybir
from concourse.bass_utils import run_bass_kernel_spmd

L, H, M = 16384, 1024, 1024
NC = 8
T = 512
S = L // (T * NC)          # 4 slots
NSL = H // 128             # 8 slices

f32 = mybir.dt.float32
f16 = mybir.dt.float16
AL = mybir.AluOpType
AX = mybir.AxisListType

_BUILD_CACHE = {}


def _build(k_list):
    key = tuple(k_list)
    if key in _BUILD_CACHE:
        return _BUILD_CACHE[key]
    nc = bacc.Bacc("TRN2", target_bir_lowering=False, debug=False,
                   num_devices=NC)

    xt_d = nc.dram_tensor("xt", [S, M, T], f16, kind="ExternalInput").ap()
    brt_d = nc.dram_tensor("brt", [M, H], f16, kind="ExternalInput").ap()
    bit_d = nc.dram_tensor("bit", [M, H], f16, kind="ExternalInput").ap()
    ct_d = nc.dram_tensor("ct", [H, M], f16, kind="ExternalInput").ap()
    cos_d = nc.dram_tensor("costb", [H, T], f16, kind="ExternalInput").ap()
    sin_d = nc.dram_tensor("sintb", [H, T], f16, kind="ExternalInput").ap()
    rc_d = nc.dram_tensor("rctb", [H, T], f16, kind="ExternalInput").ap()
    rs_d = nc.dram_tensor("rstb", [H, T], f16, kind="ExternalInput").ap()
    consts_d = nc.dram_tensor("consts", [128, 128], f32, kind="ExternalInput").ap()
    cwfre_d = nc.dram_tensor("cwfre", [128, 128], f32, kind="ExternalInput").ap()
    cwfim_d = nc.dram_tensor("cwfim", [128, 128], f32, kind="ExternalInput").ap()
    rwfre_d = nc.dram_tensor("rwfre", [128, 128], f32, kind="ExternalInput").ap()
    rwfim_d = nc.dram_tensor("rwfim", [128, 128], f32, kind="ExternalInput").ap()
    ident_d = nc.dram_tensor("ident", [128, 128], f32, kind="ExternalInput").ap()
    ddiag_d = nc.dram_tensor("ddiag", [M, 128], f16, kind="ExternalInput").ap()
    y_d = nc.dram_tensor("y", [S, M, T], f16, kind="ExternalOutput").ap()

    with tile.TileContext(nc) as tc:
        with tc.tile_pool(name="pw", bufs=1) as pw, \
             tc.tile_pool(name="px", bufs=1) as px, \
             tc.tile_pool(name="pg", bufs=1) as pg, \
             tc.tile_pool(name="pc", bufs=1) as pcp, \
             tc.tile_pool(name="pp", bufs=1, space="PSUM") as pp, \
             tc.tile_pool(name="pd", bufs=1, space="DRAM") as pd:

            brt_sb = [pw.tile([128, H], f16, name=f"brt{d}") for d in range(NSL)]
            bit_sb = [pw.tile([128, H], f16, name=f"bit{d}") for d in range(NSL)]
            ct_sb = [pw.tile([128, M], f16, name=f"ct{d}") for d in range(NSL)]
            cos_sb = [pw.tile([128, T], f16, name=f"cos{d}") for d in range(NSL)]
            sin_sb = [pw.tile([128, T], f16, name=f"sin{d}") for d in range(NSL)]
            rc_sb = [pw.tile([128, k_list[d]], f16, name=f"rc{d}")
                     for d in range(NSL)]
            rs_sb = [pw.tile([128, k_list[d]], f16, name=f"rs{d}")
                     for d in range(NSL)]
            dd_sb = [pw.tile([128, 128], f16, name=f"ddiag{d}") for d in range(NSL)]

            ident_sb = pw.tile([128, 128], f32, name="ident")
            nc.sync.dma_start(ident_sb[:], ident_d)
            cwfre_sb = pw.tile([128, 128], f32, name="cwfre")
            cwfim_sb = pw.tile([128, 128], f32, name="cwfim")
            rwfre_sb = pw.tile([128, 128], f32, name="rwfre")
            rwfim_sb = pw.tile([128, 128], f32, name="rwfim")

            def emit_deferred_tables():
                for d in range(NSL):
                    nc.sync.dma_start(brt_sb[d][:], brt_d[d * 128:(d + 1) * 128, :])
                    nc.sync.dma_start(bit_sb[d][:], bit_d[d * 128:(d + 1) * 128, :])
                for d in range(NSL):
                    nc.sync.dma_start(cos_sb[d][:], cos_d[d * 128:(d + 1) * 128, :])
                    nc.sync.dma_start(sin_sb[d][:], sin_d[d * 128:(d + 1) * 128, :])
                for d in range(NSL):
                    nc.sync.dma_start(ct_sb[d][:], ct_d[d * 128:(d + 1) * 128, :])
                    nc.sync.dma_start(rc_sb[d][:],
                                      rc_d[d * 128:(d + 1) * 128, 0:k_list[d]])
                    nc.sync.dma_start(rs_sb[d][:],
                                      rs_d[d * 128:(d + 1) * 128, 0:k_list[d]])
                nc.sync.dma_start(cwfre_sb[:], cwfre_d)
                nc.sync.dma_start(cwfim_sb[:], cwfim_d)
                nc.sync.dma_start(rwfre_sb[:], rwfre_d)
                nc.sync.dma_start(rwfim_sb[:], rwfim_d)
                for d in range(NSL):
                    nc.sync.dma_start(dd_sb[d][:], ddiag_d[d * 128:(d + 1) * 128, :])

            # consts: DMA [128,128] then transpose once -> columns
            craw = pw.tile([128, 128], f32, name="craw")
            nc.sync.dma_start(craw[:], consts_d)
            cps = pp.tile([128, 128], f32, name="cps", tag="tp", bufs=2)
            nc.tensor.transpose(cps[:], craw[:], ident_sb[:])
            cT = pw.tile([128, 128], f32, name="cT")
            nc.vector.tensor_copy(cT[:], cps[:])

            def ccv(q, pt):
                return cT[:, 16 * q + pt:16 * q + 16:2]

            def ccol(q, sl, pt):
                return cT[:, 16 * q + 2 * sl + pt:16 * q + 2 * sl + pt + 1]

            zR_re = pcp.tile([128, 8], f32, name="zR_re")
            zR_im = pcp.tile([128, 8], f32, name="zR_im")
            nc.vector.memzero(zR_re[:])
            nc.vector.memzero(zR_im[:])
            state = {"R_re": zR_re, "R_im": zR_im}
            saved = {}

            def emit_scanphase(s, after_xt=None, inserts=None):
                inserts = inserts or {}
                xt_sb = []
                for d in range(NSL):
                    t_ = px.tile([128, T], f16, name=f"xt_s{s}_d{d}",
                                 tag="xt", bufs=24)
                    nc.sync.dma_start(t_[:], xt_d[s, d * 128:(d + 1) * 128, :])
                    xt_sb.append(t_)
                if after_xt is not None:
                    after_xt()

                d_t = []
                wlre = pcp.tile([128, 8], f32, name=f"wlre{s}", tag="wl", bufs=2)
                wlim = pcp.tile([128, 8], f32, name=f"wlim{s}", tag="wl2", bufs=2)
                for sl in range(NSL):
                    for fn in inserts.get(sl, ()):
                        fn()
                    hs = slice(sl * 128, (sl + 1) * 128)
                    ps_re = pp.tile([128, T], f32, name=f"psre{s}_{sl}",
                                    tag="bure", bufs=2)
                    ps_im = pp.tile([128, T], f32, name=f"psim{s}_{sl}",
                                    tag="buim", bufs=2)
                    for d in range(NSL):
                        nc.tensor.matmul(ps_re[:], brt_sb[d][:, hs], xt_sb[d][:],
                                         start=(d == 0), stop=(d == NSL - 1))
                    for d in range(NSL):
                        nc.tensor.matmul(ps_im[:], bit_sb[d][:, hs], xt_sb[d][:],
                                         start=(d == 0), stop=(d == NSL - 1))
                    pre = pg.tile([128, T], f16, name=f"pre{s}_{sl}", tag="pre", bufs=3)
                    pim = pg.tile([128, T], f16, name=f"pim{s}_{sl}", tag="pim", bufs=3)
                    nc.scalar.copy(pre[:], ps_re[:])
                    nc.scalar.copy(pim[:], ps_im[:])
                    t0 = pg.tile([128, T], f16, name=f"t0_{s}_{sl}", tag="t0", bufs=2)
                    t1 = pg.tile([128, T], f16, name=f"t1_{s}_{sl}", tag="t1", bufs=2)
                    t2 = pg.tile([128, T], f16, name=f"t2_{s}_{sl}", tag="t2", bufs=2)
                    t3 = pg.tile([128, T], f16, name=f"t3_{s}_{sl}", tag="t3", bufs=2)
                    gre = pg.tile([128, T], f16, name=f"gre_{s}_{sl}", tag="gre", bufs=2)
                    gim = pg.tile([128, T], f16, name=f"gim_{s}_{sl}", tag="gim", bufs=2)
                    nc.vector.tensor_tensor(t0[:], cos_sb[sl][:], pre[:], AL.mult)
                    nc.vector.tensor_tensor(t1[:], sin_sb[sl][:], pim[:], AL.mult)
                    nc.vector.tensor_add(gre[:], t0[:], t1[:])
                    nc.vector.tensor_tensor(t2[:], cos_sb[sl][:], pim[:], AL.mult)
                    nc.vector.tensor_tensor(t3[:], sin_sb[sl][:], pre[:], AL.mult)
                    nc.vector.tensor_sub(gim[:], t2[:], t3[:])
                    wre = pg.tile([128, T], f16, name=f"wre_{s}_{sl}", tag="wre", bufs=3)
                    wim = pg.tile([128, T], f16, name=f"wim_{s}_{sl}", tag="wim", bufs=3)
                    rdec = ccol(5, sl, 1).broadcast_to([128, T])
                    nc.vector.tensor_tensor_scan(wre[:], rdec, gre[:],
                                                 0.0, AL.mult, AL.add)
                    nc.vector.tensor_tensor_scan(wim[:], rdec, gim[:],
                                                 0.0, AL.mult, AL.add)
                    p0 = pg.tile([128, T], f16, name=f"p0_{s}_{sl}", tag="p0", bufs=2)
                    p1 = pg.tile([128, T], f16, name=f"p1_{s}_{sl}", tag="p1", bufs=2)
                    nc.vector.tensor_tensor(p0[:], cos_sb[sl][:], wre[:], AL.mult)
                    nc.vector.tensor_tensor(p1[:], sin_sb[sl][:], wim[:], AL.mult)
                    dt = pg.tile([128, T], f16, name=f"d{s}_{sl}", tag="dd", bufs=24)
                    nc.vector.tensor_sub(dt[:], p0[:], p1[:])
                    d_t.append(dt)
                    nc.scalar.copy(wlre[:, sl:sl + 1], wre[:, T - 1:T])
                    nc.scalar.copy(wlim[:, sl:sl + 1], wim[:, T - 1:T])

                # E = ROTT1 * W_last  (standalone chunk sum) on Pool
                epack = pcp.tile([128, 16], f32, name=f"epack{s}", tag="ep", bufs=2)
                sa = pcp.tile([128, 8], f32, name=f"sa{s}", tag="sa", bufs=2)
                sb_ = pcp.tile([128, 8], f32, name=f"sb{s}", tag="sb", bufs=2)
                sc_ = pcp.tile([128, 8], f32, name=f"sc{s}", tag="sc", bufs=2)
                sd = pcp.tile([128, 8], f32, name=f"sd{s}", tag="sd", bufs=2)
                nc.gpsimd.tensor_tensor(sa[:], ccv(2, 0), wlre[:], AL.mult)
                nc.gpsimd.tensor_tensor(sb_[:], ccv(2, 1), wlim[:], AL.mult)
                nc.gpsimd.tensor_tensor(epack[:, 0:16:2], sa[:], sb_[:], AL.subtract)
                nc.gpsimd.tensor_tensor(sc_[:], ccv(2, 0), wlim[:], AL.mult)
                nc.gpsimd.tensor_tensor(sd[:], ccv(2, 1), wlre[:], AL.mult)
                nc.gpsimd.tensor_tensor(epack[:, 1:16:2], sc_[:], sd[:], AL.add)

                saved[s] = dict(xt_sb=xt_sb, d_t=d_t, epack=epack)

            def emit_pub(s):
                sv = saved[s]
                pub_ps = pp.tile([16, 128], f32, name=f"pubps{s}", tag="tp", bufs=2)
                nc.tensor.transpose(pub_ps[:], sv["epack"][:], ident_sb[:])
                pub_sb = pcp.tile([16, 128], f32, name=f"pubsb{s}", tag="pub", bufs=2)
                nc.vector.tensor_copy(pub_sb[:], pub_ps[:])
                pub_dr = pd.tile([16, 128], f32, name=f"pubdr{s}", tag="pubd", bufs=2)
                nc.sync.dma_start(pub_dr[:], pub_sb[:])
                gat_dr = pd.tile([128, 128], f32, name=f"gatdr{s}", tag="gatd",
                                 bufs=2, addr_space="Shared")
                nc.gpsimd.collective_compute(
                    "AllGather", AL.bypass,
                    replica_groups=[list(range(NC))],
                    ins=[pub_dr[:].opt()],
                    outs=[gat_dr[:].opt()],
                )
                sv["gat_dr"] = gat_dr

            def emit_ktr(s):
                # gather -> SBUF -> PE transpose -> Act copy to SBUF
                sv = saved[s]
                eg = pcp.tile([128, 128], f32, name=f"eg{s}", tag="eg", bufs=2)
                nc.sync.dma_start(eg[:], sv["gat_dr"][:])
                et_ps = pp.tile([128, 128], f32, name=f"etps{s}", tag="tp", bufs=2)
                nc.tensor.transpose(et_ps[:], eg[:], ident_sb[:])
                et = pcp.tile([128, 128], f32, name=f"et{s}", tag="et", bufs=2)
                nc.scalar.copy(et[:], et_ps[:])
                sv["et"] = et

            def emit_kmath(s):
                sv = saved[s]
                et = sv.pop("et")

                def wsum(fold_sb, nmv, nmr1, nm):
                    tmp = pcp.tile([128, 128], f32, name=f"{nmv}{s}", tag="redt", bufs=2)
                    nc.vector.tensor_tensor(tmp[:], fold_sb[:], et[:], AL.mult)
                    red1 = pcp.tile([128, 16], f32, name=f"{nmr1}{s}", tag="red1", bufs=2)
                    nc.vector.tensor_reduce(
                        red1[:].unsqueeze(2),
                        tmp[:].rearrange("p (j x) -> p x j", j=8),
                        AX.X, AL.add)
                    out = pcp.tile([128, 8], f32, name=f"{nm}{s}", tag=nm, bufs=2)
                    nc.vector.tensor_reduce(
                        out[:].unsqueeze(2),
                        red1[:].rearrange("p (sl pt) -> p sl pt", pt=2),
                        AX.X, AL.add)
                    return out

                v_re = wsum(cwfre_sb, "tmpa", "reda", "vre")
                v_im = wsum(cwfim_sb, "tmpb", "redb", "vim")
                rp_re = wsum(rwfre_sb, "tmpc", "redc", "rpre")
                rp_im = wsum(rwfim_sb, "tmpd", "redd", "rpim")

                _sc = [0]

                def t8(a, b, op):
                    _sc[0] += 1
                    out = pcp.tile([128, 8], f32, name=f"cs{s}_{_sc[0]}",
                                   tag=f"cs{_sc[0] % 12}", bufs=2)
                    nc.gpsimd.tensor_tensor(out[:], a, b, op)
                    return out[:]

                def cmul(wre_v, wim_v, zre, zim):
                    re = t8(t8(wre_v, zre, AL.mult), t8(wim_v, zim, AL.mult),
                            AL.subtract)
                    im = t8(t8(wre_v, zim, AL.mult), t8(wim_v, zre, AL.mult),
                            AL.add)
                    return re, im

                R_re, R_im = state["R_re"], state["R_im"]
                qr_re, qr_im = cmul(ccv(0, 0), ccv(0, 1), R_re[:], R_im[:])
                vt_re = t8(v_re[:], qr_re, AL.add)
                vt_im = t8(v_im[:], qr_im, AL.add)
                vpre_t = pcp.tile([128, 8], f32, name=f"vpre{s}", tag="vp1", bufs=2)
                vpim_t = pcp.tile([128, 8], f32, name=f"vpim{s}", tag="vp2", bufs=2)
                ra = t8(ccv(3, 0), vt_re, AL.mult)
                rb = t8(ccv(3, 1), vt_im, AL.mult)
                nc.gpsimd.tensor_tensor(vpre_t[:], ra, rb, AL.subtract)
                rcx = t8(ccv(3, 0), vt_im, AL.mult)
                rd = t8(ccv(3, 1), vt_re, AL.mult)
                nc.gpsimd.tensor_tensor(vpim_t[:], rcx, rd, AL.add)
                q8r_re, q8r_im = cmul(ccv(4, 0), ccv(4, 1), R_re[:], R_im[:])
                rn_re = pcp.tile([128, 8], f32, name=f"rnre{s}", tag="rn", bufs=2)
                rn_im = pcp.tile([128, 8], f32, name=f"rnim{s}", tag="rn2", bufs=2)
                nc.gpsimd.tensor_tensor(rn_re[:], q8r_re, rp_re[:], AL.add)
                nc.gpsimd.tensor_tensor(rn_im[:], q8r_im, rp_im[:], AL.add)
                state["R_re"], state["R_im"] = rn_re, rn_im
                sv["vp"] = (vpre_t, vpim_t)

            def emit_kz(s, sls):
                # u[sl] = d[sl] + (RC*vp_re - RS*vp_im) on a K-prefix only
                # (channels are r-sorted so the correction support shrinks)
                sv = saved[s]
                vpre_t, vpim_t = sv["vp"]
                u_t = sv.setdefault("u_t", [None] * NSL)
                for sl in sls:
                    K = k_list[sl]
                    z1 = pg.tile([128, K], f16, name=f"z1_{s}_{sl}", tag="z1", bufs=4)
                    z2 = pg.tile([128, K], f16, name=f"z2_{s}_{sl}", tag="z2", bufs=4)
                    nc.vector.tensor_scalar_mul(z1[:], rc_sb[sl][:],
                                                vpre_t[:, sl:sl + 1])
                    nc.vector.tensor_scalar_mul(z2[:], rs_sb[sl][:],
                                                vpim_t[:, sl:sl + 1])
                    zz = pg.tile([128, K], f16, name=f"zz{s}_{sl}", tag="zz", bufs=4)
                    nc.gpsimd.tensor_tensor(zz[:], z1[:], z2[:], AL.subtract)
                    dsl = sv["d_t"][sl]
                    u = pg.tile([128, T], f16, name=f"u{s}_{sl}", tag="u", bufs=9)
                    nc.gpsimd.tensor_tensor(u[:, 0:K], dsl[:, 0:K], zz[:], AL.add)
                    if K < T:
                        nc.gpsimd.tensor_copy(u[:, K:T], dsl[:, K:T])
                    u_t[sl] = u

            def emit_cgemm(s):
                sv = saved.pop(s)
                xt_sb = sv["xt_sb"]
                u_t = sv["u_t"]
                # slice 0's (full-width) correction lands last; accumulate it
                # last so the PE starts each psy without waiting on it
                order = list(range(1, NSL)) + [0]
                for n in range(NSL):
                    ns = slice(n * 128, (n + 1) * 128)
                    psy = pp.tile([128, T], f32, name=f"psy{s}_{n}", tag="ytile",
                                  bufs=2)
                    nc.tensor.matmul(psy[:], dd_sb[n][:], xt_sb[n][:],
                                     start=True, stop=False)
                    for i, sl in enumerate(order):
                        nc.tensor.matmul(psy[:], ct_sb[sl][:, ns], u_t[sl][:],
                                         start=False, stop=(i == NSL - 1))
                    yo = pg.tile([128, T], f16, name=f"yo{s}_{n}", tag="yo", bufs=3)
                    nc.scalar.copy(yo[:], psy[:])
                    nc.sync.dma_start(y_d[s, ns, :], yo[:])

            # chunk-s carry pipeline runs entirely inside scanphase(s+1):
            # pub@sl0, gather-transpose@sl4, carry-math@sl5, corrections
            # (slices 1..7 then 0) @sl6/7; C(s) follows immediately.
            def carry_inserts(s):
                return {0: [lambda: emit_pub(s)],
                        4: [lambda: emit_ktr(s)],
                        5: [lambda: emit_kmath(s)],
                        6: [lambda: emit_kz(s, list(range(1, NSL)))],
                        7: [lambda: emit_kz(s, [0])]}

            emit_scanphase(0, after_xt=emit_deferred_tables)
            emit_scanphase(1, inserts=carry_inserts(0))
            emit_cgemm(0)
            emit_scanphase(2, inserts=carry_inserts(1))
            emit_cgemm(1)
            emit_scanphase(3, inserts=carry_inserts(2))
            emit_cgemm(2)
            emit_pub(3)
            emit_ktr(3)
            emit_kmath(3)
            emit_kz(3, list(range(1, NSL)))
            emit_kz(3, [0])
            emit_cgemm(3)

    nc.compile()
    _BUILD_CACHE[key] = nc
    return nc


def _prep(inputs, A_re, A_im, B_re, B_im, C, D):
    x = np.asarray(inputs, dtype=np.float32)
    A_re = np.asarray(A_re, dtype=np.float32)
    A_im = np.asarray(A_im, dtype=np.float32)
    B_re = np.asarray(B_re, dtype=np.float32)
    B_im = np.asarray(B_im, dtype=np.float32)
    C = np.asarray(C, dtype=np.float32)
    D = np.asarray(D, dtype=np.float32)
    A = A_re.astype(np.float64) + 1j * A_im.astype(np.float64)
    # sort channels by |A| descending so correction prefixes shrink per slice
    perm = np.argsort(-np.abs(A), kind="stable")
    A = A[perm]
    B_re = B_re[perm, :]
    B_im = B_im[perm, :]
    C = C[:, perm]
    r = np.abs(A)
    th = np.angle(A)
    # per-slice correction prefix: r^(K+1) < 1e-3 for every channel in slice
    k_list = []
    for sl in range(NSL):
        rmax = r[sl * 128:(sl + 1) * 128].max()
        if rmax >= 0.999:
            K = T
        else:
            K = int(np.ceil(np.log(1e-3) / np.log(rmax)))
            K = min(T, max(8, ((K + 7) // 8) * 8))
        k_list.append(K)
    k = np.arange(T)
    COS = np.cos(th[:, None] * k)
    SIN = np.sin(th[:, None] * k)
    RPOW = r[:, None] ** (k + 1)
    Q = A ** T
    ROT1 = np.exp(1j * th)
    ROTT1 = np.exp(1j * th * (T - 1))
    Q8 = Q ** 8
    RW = [Q ** (7 - j) for j in range(NC)]

    brt = np.ascontiguousarray(B_re.T).astype(np.float16)
    bit = np.ascontiguousarray(B_im.T).astype(np.float16)
    ct = np.ascontiguousarray(C.T).astype(np.float16)
    cos_t = COS.astype(np.float16)
    sin_t = SIN.astype(np.float16)
    rc = (RPOW * COS).astype(np.float16)
    rs = (RPOW * SIN).astype(np.float16)
    ident = np.eye(128, dtype=np.float32)
    ddiag = np.zeros((M, 128), np.float16)
    for n in range(NSL):
        ddiag[n * 128:(n + 1) * 128, :] = np.diag(D[n * 128:(n + 1) * 128])

    xT = np.ascontiguousarray(x.T)  # [M, L]

    def cvec_rows(z):
        out = np.zeros((16, 128), np.float32)
        zr = z.real.astype(np.float32).reshape(8, 128)
        zi = z.imag.astype(np.float32).reshape(8, 128)
        out[0::2] = zr
        out[1::2] = zi
        return out

    rwf_re = np.zeros((128, 128), np.float32)
    rwf_im = np.zeros((128, 128), np.float32)
    for j in range(NC):
        w = RW[j]
        wr = w.real.astype(np.float32).reshape(8, 128)
        wi = w.imag.astype(np.float32).reshape(8, 128)
        for sl in range(8):
            rwf_re[:, 16 * j + 2 * sl + 0] = wr[sl]
            rwf_re[:, 16 * j + 2 * sl + 1] = -wi[sl]
            rwf_im[:, 16 * j + 2 * sl + 0] = wi[sl]
            rwf_im[:, 16 * j + 2 * sl + 1] = wr[sl]

    in_maps = []
    for c in range(NC):
        QPC = Q ** c
        consts = np.zeros((128, 128), np.float32)
        consts[0:16] = cvec_rows(QPC)
        consts[16:32] = cvec_rows(Q ** (c + 1))
        consts[32:48] = cvec_rows(ROTT1)
        consts[48:64] = cvec_rows(ROT1)
        consts[64:80] = cvec_rows(Q8)
        consts[80:96] = cvec_rows(D.astype(np.float64) + 1j * r)

        cwf_re = np.zeros((128, 128), np.float32)
        cwf_im = np.zeros((128, 128), np.float32)
        for j in range(c):
            w = Q ** (c - 1 - j)
            wr = w.real.astype(np.float32).reshape(8, 128)
            wi = w.imag.astype(np.float32).reshape(8, 128)
            for sl in range(8):
                cwf_re[:, 16 * j + 2 * sl + 0] = wr[sl]
                cwf_re[:, 16 * j + 2 * sl + 1] = -wi[sl]
                cwf_im[:, 16 * j + 2 * sl + 0] = wi[sl]
                cwf_im[:, 16 * j + 2 * sl + 1] = wr[sl]

        xt = np.zeros((S, M, T), np.float16)
        for s in range(S):
            m = 8 * s + c
            xt[s] = xT[:, m * T:(m + 1) * T]

        in_maps.append({
            "xt": xt, "brt": brt, "bit": bit, "ct": ct,
            "costb": cos_t, "sintb": sin_t, "rctb": rc, "rstb": rs,
            "consts": consts,
            "cwfre": cwf_re, "cwfim": cwf_im,
            "rwfre": rwf_re, "rwfim": rwf_im,
            "ident": ident, "ddiag": ddiag,
        })
    return in_maps, k_list


LAST_RESULTS = {}


def kernel(inputs, A_re, A_im, B_re, B_im, C, D):
    in_maps, k_list = _prep(inputs, A_re, A_im, B_re, B_im, C, D)
    nc = _build(k_list)
    trace = bool(os.environ.get("DIAG_TRACE"))
    res = run_bass_kernel_spmd(nc, in_maps, core_ids=list(range(NC)),
                               trace=trace)
    LAST_RESULTS["exec_time_ns"] = res.exec_time_ns
    LAST_RESULTS["mean_exec_time_ns"] = res.mean_exec_time_ns
    yT = np.zeros((M, L), np.float32)
    for c in range(NC):
        yc = res.results[c]["y"].astype(np.float32)
        for s in range(S):
            m = 8 * s + c
            yT[:, m * T:(m + 1) * T] = yc[s]
    return np.ascontiguousarray(yT.T)
